# revision 15
# baseline (speedup 1.0000x reference)
"""AttentionSink masked-add kernel for 8 TRN2 NeuronCores.

out[b,h,i,j] = w[b,h,i,j] + mask[i,j], mask 0 where allowed else -1e30.
Allowed: j < 4 (sink) or i-25 <= j <= i (local band).

Since |w| << ulp(-1e30) in fp32, masked outputs are exactly -1e30. The
kernel therefore:
  1. writes the whole output with the constant -1e30 from a small SBUF tile
     (stride-0 broadcast DMA source, full 8 KiB rows, ~peak HBM write BW);
  2. overwrites the allowed positions by copying them straight from the
     input: the sink columns as a thin DRAM->DRAM copy, and the local band
     as a DRAM->DRAM copy over a diagonal access pattern (stride S+1), which
     covers exactly the 26-wide allowed parallelogram for row blocks r>=1 —
     no arithmetic needed since the mask is 0 there.
Only block r=0 (rows 0..127, where the band clips at column 0) goes through
SBUF with a real mask add. Total HBM traffic per core: ~134 MB written +
~11 MB read, ~1.5% of the input read.

The 64 (S,S) matrices are split 8 per core; no collectives.
"""

import sys

import numpy as np

try:
    import concourse.bass as bass
except ImportError:  # fresh environment: add the repo staging paths
    for p in ("/opt/trn_rl_repo", "/root/.axon_site/_ro/trn_rl_repo"):
        if p not in sys.path:
            sys.path.append(p)
    import concourse.bass as bass

import concourse.tile as tile
from concourse import bacc, mybir
from concourse.bass_utils import run_bass_kernel_spmd

B, H, S = 4, 16, 2048
SINK = 4
LEFT = 25
NEG = -1e30
P = 128                    # SBUF partitions / rows per block
NBLK = S // P              # 16 row blocks per matrix
N_CORES = 8
M = (B * H) // N_CORES     # matrices per core


def _host_masks():
    # mask for rows 0..127 x cols 0..127 (sink + clamped band; block 0)
    i = np.arange(P)[:, None]
    j = np.arange(P)[None, :]
    allowed0 = (j < SINK) | ((j >= i - LEFT) & (j <= i))
    return np.where(allowed0, 0.0, NEG).astype(np.float32)


def _build_program(repeat=1):
    nc = bacc.Bacc(
        "TRN2", target_bir_lowering=False, debug=False, num_devices=N_CORES
    )
    dt = mybir.dt.float32
    x = nc.dram_tensor("x", [M, S, S], dt, kind="ExternalInput").ap()
    mf = nc.dram_tensor("mask_first", [P, P], dt, kind="ExternalInput").ap()
    out = nc.dram_tensor("out", [M, S, S], dt, kind="ExternalOutput").ap()

    def bcast_m(ap2d, m=M):
        # (p, w) SBUF AP -> (p, m, w) with stride-0 middle dim
        (ps, pn), (ws, wn) = ap2d.ap
        return bass.AP(ap2d.tensor, ap2d.offset, [[ps, pn], [0, m], [ws, wn]])

    with tile.TileContext(nc) as tc:
        with tc.tile_pool(name="pool", bufs=1) as pool:
            # constant -1e30 background row, split memset across two engines
            c = pool.tile([P, S], dt, name="c")
            nc.vector.memset(c[:, 0 : S * 5 // 9], NEG)
            nc.gpsimd.memset(c[:, S * 5 // 9 : S], NEG)

            # block-0 mask and band data
            mf_t = pool.tile([P, P], dt, name="mf_t")
            nc.gpsimd.dma_start(mf_t[:], mf[:])
            bt0 = pool.tile([P, M, P], dt, name="bt0")
            nc.gpsimd.dma_start(
                bt0[:], x[:, 0:P, 0:P].rearrange("m p w -> p m w")
            )
            nc.vector.tensor_add(bt0[:], bt0[:], bcast_m(mf_t[:]))

            for _rep in range(repeat):
              # Block 0 is emitted LAST: the band copy below overlaps the
              # r>=1 const stores but not block 0's, so its post-wait
              # descriptor prep hides behind block 0's transfer.
              for r in list(range(1, NBLK)) + [0]:
                R = r * P
                # constant background store (near-8 KiB contiguous rows);
                # alternates the two HWDGE rings so transfers pipeline
                # back-to-back. Starts at col 4 (rows >= 128) / col 128
                # (block 0) so the sink / block-0 stores below overlap no
                # const store and can dispatch without waiting.
                lo = P if r == 0 else SINK
                ceng = nc.sync if r % 2 == 0 else nc.scalar
                ceng.dma_start(
                    out[:, R : R + P, lo:S].rearrange("m p c -> p m c"),
                    bcast_m(c[:, lo:S]),
                )

              # rows 0..127, cols 0..127: computed sink+clamped-band block via
              # SBUF (SWDGE; no dependency on any const store)
              nc.gpsimd.dma_start(
                  out[:, 0:P, 0:P].rearrange("m p w -> p m w"), bt0[:]
              )
              # sink columns rows 128..2047: one thin DRAM->DRAM passthrough
              nc.sync.dma_start(
                  out[:, P:S, 0:SINK], x[:, P:S, 0:SINK]
              )
              # band rows 128..2047: one DRAM->DRAM copy over the diagonal
              # parallelograms: out[m, r*128+p, r*128-25+p+q], q in [0, 26)
              off = P * S + (P - LEFT)
              dims = [
                  [S * S, M],
                  [P * (S + 1), NBLK - 1],
                  [S + 1, P],
                  [1, LEFT + 1],
              ]
              nc.scalar.dma_start(
                  bass.AP(out.tensor, off, dims),
                  bass.AP(x.tensor, off, dims),
              )

    nc.compile()
    return nc


_CACHE = {}


def _get_nc():
    if "nc" not in _CACHE:
        _CACHE["nc"] = _build_program()
    return _CACHE["nc"]


def _in_maps(w):
    mask_first = _host_masks()
    flat = w.reshape(B * H, S, S)
    return [
        {"x": flat[i * M : (i + 1) * M], "mask_first": mask_first}
        for i in range(N_CORES)
    ]


def _gather(chunks):
    """Stack per-core (M,S,S) results along axis 0. Zero-copy when they are
    consecutive contiguous slices of one base buffer (bass2jax returns views
    of a single concatenated array); otherwise fall back to a copy."""
    try:
        c0 = chunks[0]
        step = c0.nbytes
        ptr0 = c0.__array_interface__["data"][0]
        base = c0.base
        if base is not None and all(
            c.base is base
            and c.flags["C_CONTIGUOUS"]
            and c.__array_interface__["data"][0] == ptr0 + i * step
            for i, c in enumerate(chunks)
        ):
            # one shared owner + adjacent layout: a strided view over c0
            # (whose .base keeps the owner alive) covers all of them
            return np.lib.stride_tricks.as_strided(
                c0,
                shape=(len(chunks),) + c0.shape,
                strides=(step,) + c0.strides,
            )
    except Exception:
        pass
    return np.concatenate([c[None] for c in chunks], axis=0)


def kernel(attention_weights, seq_len=None):
    w = np.ascontiguousarray(np.asarray(attention_weights, dtype=np.float32))
    assert w.shape == (B, H, S, S)
    nc = _get_nc()
    in_maps = _in_maps(w)
    res = run_bass_kernel_spmd(nc, in_maps, core_ids=list(range(N_CORES)))
    out = _gather([res.results[i]["out"] for i in range(N_CORES)])
    return out.reshape(B, H, S, S)
